# revision 24
# baseline (speedup 1.0000x reference)
"""Trainium2 Bass kernel for sparse causal attention (nn_CausalAttentionKV).

Reference computation (fp32, single device):
    q_all = x @ Wq + bq ; k_all = x @ Wk + bk ; v_all = x @ Wv + bv
    q = gather(q_all, query_idx)        # (B, M, D) selected query rows
    att = softmax(mask(q k^T / sqrt(hd)))   # per-query causal mask t <= qidx[m]
    y = (att v) @ Wo + bo

Shapes: B=4, T=4096, D=2048, n_head=16, hd=128, M=512.

Sharding (8 cores): core = 2*b + g  handles batch b and head-group g
(8 heads = 1024 feature cols).  Q/K/V projections are column-parallel,
out-proj is row-parallel; the two partial outputs per batch are summed
on the host.  All matmul inputs are bf16 (fp32 PSUM accumulation).

Host-side prep per core: transpose x/xq to (D, T) layout (the PE needs
the contraction dim on partitions), gather the M query rows of x,
slice/cast weights, precompute the additive causal mask, and compute
per-t-block skip bounds from the (sorted) query_idx so fully-masked
regions of the score matrix are never computed.

Per-core schedule: Q projection first (small, covers the K/V weight
prefetch), then one fused pass over x computing K^T (streamed out to a
DRAM scratch) and V (resident in SBUF), then per-head attention with
K^T streamed back in, then the output projection.
"""

import sys
import types
from contextlib import ExitStack

import numpy as np
import ml_dtypes

import concourse.bass as bass
import concourse.tile as tile
import concourse.mybir as mybir
from concourse import bacc
from concourse.bass_utils import run_bass_kernel_spmd

BF16 = mybir.dt.bfloat16
F32 = mybir.dt.float32
NPBF = ml_dtypes.bfloat16

B, T, D = 4, 4096, 2048
NH, HD, M = 16, 128, 512
NHG = 8            # heads per core (group)
DG = NHG * HD      # 1024 feature cols per core
NT = T // 128      # 32 t-chunks
ND = D // 128      # 16 d-chunks
MASK_VAL = np.float32(-30000.0)


def _install_ntff_hook():
    """Register the axon NTFF profiling hook if the image's antenv lacks it."""
    try:
        from antenv.axon_hooks import get_axon_ntff_profile_hook  # noqa: F401
        return
    except ImportError:
        pass
    try:
        import antenv
        from trn_agent_boot.trn_boot import _ntff_profile_via_ctypes

        mod = types.ModuleType("antenv.axon_hooks")
        hook = [None]
        mod.set_axon_ntff_profile_hook = lambda h: hook.__setitem__(0, h)
        mod.get_axon_ntff_profile_hook = lambda: hook[0]
        sys.modules["antenv.axon_hooks"] = mod
        antenv.axon_hooks = mod
        mod.set_axon_ntff_profile_hook(
            _ntff_profile_via_ctypes("/opt/axon/libaxon_pjrt.so")
        )
    except Exception:
        pass


def build_program(flo, fhi):
    """Build the per-core Bass program.

    flo[i]: first m column with any allowed key in t-chunk i (cols below
            are fully masked there -> never computed).
    fhi[i]: first m column fully allowed in t-chunk i (cols beyond need
            no mask add).
    Both are unions over the 4 batches so one program serves all cores.
    """
    nc = bacc.Bacc("TRN2", target_bir_lowering=False, debug=False)

    xT = nc.dram_tensor("xT", [D, T], BF16, kind="ExternalInput")
    xqT = nc.dram_tensor("xqT", [D, M], BF16, kind="ExternalInput")
    wk = nc.dram_tensor("wk", [D, DG], BF16, kind="ExternalInput")
    wv = nc.dram_tensor("wv", [D, DG], BF16, kind="ExternalInput")
    wq = nc.dram_tensor("wq", [D, DG], BF16, kind="ExternalInput")
    wo = nc.dram_tensor("wo", [DG, D], BF16, kind="ExternalInput")
    maskd = nc.dram_tensor("mask", [T, M], BF16, kind="ExternalInput")
    bks = nc.dram_tensor("bks", [128, NHG], F32, kind="ExternalInput")
    bqs = nc.dram_tensor("bqs", [128, NHG], F32, kind="ExternalInput")
    y = nc.dram_tensor("y", [M, D], F32, kind="ExternalOutput")

    # (c*128+p, t) views for 4-chunk batched DMA
    xTr = xT.rearrange("(c p) t -> p c t", p=128)
    xqTr = xqT.rearrange("(c p) t -> p c t", p=128)
    wkr = wk.rearrange("(c p) t -> p c t", p=128)
    wvr = wv.rearrange("(c p) t -> p c t", p=128)
    wqr = wq.rearrange("(c p) t -> p c t", p=128)
    wor = wo.rearrange("(c p) t -> p c t", p=128)
    maskr = maskd.rearrange("(c p) t -> p c t", p=128)

    with ExitStack() as ctx:
        tc = ctx.enter_context(tile.TileContext(nc))

        # ---- persistent tiles --------------------------------------
        persist = ctx.enter_context(tc.tile_pool(name="persist", bufs=1))
        v_t = [persist.tile([128, DG], BF16, name=f"v{i}", tag=f"v{i}") for i in range(NT)]
        qt_t = [persist.tile([128, M], BF16, name=f"qt{j}", tag=f"qt{j}") for j in range(NHG)]
        ot_t = [persist.tile([128, M], BF16, name=f"ot{j}", tag=f"ot{j}") for j in range(NHG)]
        bias_k = persist.tile([128, NHG], F32, name="bias_k", tag="bias_k")
        bias_q = persist.tile([128, NHG], F32, name="bias_q", tag="bias_q")
        zbias = persist.tile([128, 1], F32, name="zbias", tag="zbias")
        ones_c = persist.tile([128, 1], BF16, name="ones_c", tag="ones_c")
        ones_r = persist.tile([1, 128], F32, name="ones_r", tag="ones_r")
        # mask super-tiles: 4 t-chunks each, shared col-window
        mlo = [min(flo[4 * g : 4 * g + 4]) for g in range(NT // 4)]
        mhi = [max(fhi[4 * g : 4 * g + 4]) for g in range(NT // 4)]
        mask_t = [
            persist.tile(
                [128, 4, max(mhi[g] - mlo[g], 1)], BF16,
                name=f"mask{g}", tag=f"mask{g}",
            )
            for g in range(NT // 4)
        ]
        dram = ctx.enter_context(tc.tile_pool(name="dram", bufs=1, space="DRAM"))
        ktd = dram.tile([NHG, 128, T], BF16, name="ktd")
        ktd_r = ktd.rearrange("j p t -> p j t")

        nc.sync.dma_start(bias_k[:], bks[:])
        nc.sync.dma_start(bias_q[:], bqs[:])
        nc.vector.memset(zbias[:], 0.0)
        nc.vector.memset(ones_c[:], 1.0)
        nc.vector.memset(ones_r[:], 1.0)

        # ---- phase A-Q: Qt[j] = ((xq @ wq_j + bq_j)/sqrt(hd))^T ----
        # wq is loaded in per-head column slices so the first matmul
        # group only waits for ~0.5 MB; runs while wk/wv prefetch.
        with (
            nc.named_scope("phase_AQ"),
            tc.tile_pool(name="wqp", bufs=1) as wqp,
            tc.tile_pool(name="xqp", bufs=1) as xqp,
            tc.tile_pool(name="pq", bufs=4, space="PSUM") as pqp,
        ):
            xq_t = [xqp.tile([128, 4, M], BF16, name=f"xq{d}", tag=f"xq{d}") for d in range(4)]
            for d in range(4):
                nc.sync.dma_start(xq_t[d][:], xqTr[:, 4 * d : 4 * d + 4, :])
            # wq in head-pair column slices: (jg, s) -> 4 d-chunks x 256 cols
            wq_t = {}
            for jg in range(4):
                for s in range(4):
                    wq_t[jg, s] = wqp.tile(
                        [128, 4, 256], BF16, name=f"wq{jg}_{s}", tag=f"wq{jg}_{s}"
                    )
                    nc.sync.dma_start(
                        wq_t[jg, s][:],
                        wqr[:, 4 * s : 4 * s + 4, jg * 256 : (jg + 1) * 256],
                    )
            inv_s = 1.0 / float(np.sqrt(HD))
            for j in range(NHG):
                jg, co = j // 2, (j % 2) * 128
                pq = pqp.tile([128, M], F32, name="pq", tag="pq")
                for d in range(ND):
                    nc.tensor.matmul(
                        pq[:],
                        wq_t[jg, d // 4][:, d % 4, co : co + 128],
                        xq_t[d // 4][:, d % 4, :],
                        start=(d == 0),
                        stop=(d == ND - 1),
                    )
                nc.scalar.activation(
                    qt_t[j][:],
                    pq[:],
                    mybir.ActivationFunctionType.Identity,
                    scale=inv_s,
                    bias=bias_q[:, j : j + 1],
                )

        # ---- phase A-KV: one pass over x computing Kt and V --------
        KTS = 512
        with (
            nc.named_scope("phase_AKV"),
            tc.tile_pool(name="wkp", bufs=1) as wkp,
            tc.tile_pool(name="wvp", bufs=1) as wvp,
            tc.tile_pool(name="xtp", bufs=2) as xtp,
            tc.tile_pool(name="kst", bufs=3) as kstp,
            tc.tile_pool(name="pk", bufs=3, space="PSUM") as pkp,
            tc.tile_pool(name="pv", bufs=3, space="PSUM") as pvp,
        ):
            wk_t = [wkp.tile([128, 4, DG], BF16, name=f"wk{d}", tag=f"wk{d}") for d in range(4)]
            wv_t = [wvp.tile([128, 4, DG], BF16, name=f"wv{d}", tag=f"wv{d}") for d in range(4)]
            for d in range(4):
                nc.sync.dma_start(wk_t[d][:], wkr[:, 4 * d : 4 * d + 4, :])
            for d in range(4):
                nc.sync.dma_start(wv_t[d][:], wvr[:, 4 * d : 4 * d + 4, :])
            for ts in range(T // KTS):
                xt_t = [xtp.tile([128, 4, KTS], BF16, name=f"xt{d}", tag=f"xt{d}") for d in range(4)]
                for d in range(4):
                    nc.sync.dma_start(
                        xt_t[d][:], xTr[:, 4 * d : 4 * d + 4, ts * KTS : (ts + 1) * KTS]
                    )
                # K^T: per head j, (hd, KTS) tile; staged 4 heads per DMA
                for jg in range(2):
                    ks = kstp.tile([128, 4, KTS], BF16, name="ks", tag="ks")
                    for jj in range(4):
                        j = 4 * jg + jj
                        pk = pkp.tile([128, KTS], F32, name="pk", tag="pk")
                        for d in range(ND):
                            nc.tensor.matmul(
                                pk[:],
                                wk_t[d // 4][:, d % 4, j * 128 : (j + 1) * 128],
                                xt_t[d // 4][:, d % 4, :],
                                start=(d == 0),
                                stop=(d == ND - 1),
                            )
                        nc.scalar.activation(
                            ks[:, jj, :],
                            pk[:],
                            mybir.ActivationFunctionType.Identity,
                            bias=bias_k[:, j : j + 1],
                        )
                    nc.sync.dma_start(
                        ktd_r[:, 4 * jg : 4 * jg + 4, ts * KTS : (ts + 1) * KTS],
                        ks[:],
                    )
                # V: (t, DG) tiles
                for u in range(KTS // 128):
                    i = ts * (KTS // 128) + u
                    for f in range(2):
                        pv = pvp.tile([128, 512], F32, name="pv", tag="pv")
                        for d in range(ND):
                            nc.tensor.matmul(
                                pv[:],
                                xt_t[d // 4][:, d % 4, u * 128 : (u + 1) * 128],
                                wv_t[d // 4][:, d % 4, f * 512 : (f + 1) * 512],
                                start=(d == 0),
                                stop=(d == ND - 1),
                            )
                        nc.vector.tensor_copy(
                            v_t[i][:, f * 512 : (f + 1) * 512], pv[:]
                        )

        # ---- phase B prefetch: out-proj weights + masks ------------
        wop = ctx.enter_context(tc.tile_pool(name="wop", bufs=1))
        wo_t = [wop.tile([128, 4, D], BF16, name=f"wo{d}", tag=f"wo{d}") for d in range(2)]
        for d in range(2):
            nc.sync.dma_start(wo_t[d][:], wor[:, 4 * d : 4 * d + 4, :])
        for g in range(NT // 4):
            if mlo[g] < M and mhi[g] > mlo[g]:
                nc.sync.dma_start(
                    mask_t[g][:, :, : mhi[g] - mlo[g]],
                    maskr[:, 4 * g : 4 * g + 4, mlo[g] : mhi[g]],
                )

        # ---- phase B: attention per head, 4-chunk batched ----------
        chunks = [i for i in range(NT) if flo[i] < M]
        pairs = [chunks[k : k + 2] for k in range(0, len(chunks), 2)]
        batches = [pairs[k : k + 2] for k in range(0, len(pairs), 2)]
        with (
            nc.named_scope("phase_B"),
            tc.tile_pool(name="kth", bufs=2) as kthp,
            tc.tile_pool(name="ps", bufs=2, space="PSUM") as psp,
            tc.tile_pool(name="po", bufs=2, space="PSUM") as pop,
            tc.tile_pool(name="pl", bufs=2, space="PSUM") as plp,
            tc.tile_pool(name="esb", bufs=6) as esb,
            tc.tile_pool(name="lsb", bufs=2) as lsb,
        ):
            po_q, pl_q = {}, {}

            def emit_norm(j):
                """Normalize head j: ot[j] = po[j] / l[j] (off PE critical path).

                l is broadcast across partitions on the PE first, so the
                reciprocal runs 128 lanes wide instead of on 1 partition.
                """
                po, pl = po_q.pop(j), pl_q.pop(j)
                l_sb = lsb.tile([1, M], F32, name="l", tag="l")
                linv = lsb.tile([1, M], F32, name="linv", tag="linv")
                nc.vector.tensor_copy(l_sb[:], pl[:])
                nc.vector.reciprocal(linv[:], l_sb[:])
                pb = psp.tile([128, M], F32, name="pb", tag="ps")
                nc.tensor.matmul(pb[:], ones_r[:], linv[:], start=True, stop=True)
                lb_sb = lsb.tile([128, M], F32, name="lb", tag="lb")
                nc.scalar.copy(lb_sb[:], pb[:])
                nc.vector.tensor_mul(ot_t[j][:], po[:], lb_sb[:])

            kth = {}
            for j in range(NHG):
                if j == 0:
                    kth[0] = kthp.tile([128, T], BF16, name="kth", tag="kth")
                    nc.sync.dma_start(kth[0][:], ktd[0])
                po = pop.tile([128, M], F32, name="po", tag="po")
                pl = plp.tile([1, M], F32, name="pl", tag="pl")
                po_q[j], pl_q[j] = po, pl
                pend = None  # previous batch: list of (i, e, u, lo)
                first = True
                lfirst = True
                for bi, batch in enumerate(batches):
                    cur = []
                    for pair in batch:
                        lo_min = min(flo[i] for i in pair)
                        pst = psp.tile([128, 2, M], F32, name="pst", tag="ps")
                        for u, i in enumerate(pair):
                            nc.tensor.matmul(
                                pst[:, u, flo[i] : M],
                                kth[j][:, i * 128 : (i + 1) * 128],
                                qt_t[j][:, flo[i] : M],
                                start=True,
                                stop=True,
                                skip_group_check=True,
                            )
                        # one mask add + one exp per pair; mask cols beyond a
                        # chunk's own [lo, fhi) window add 0 or touch lanes
                        # the narrower chunk never reads
                        fhi_max = max(fhi[i] for i in pair)
                        g = pair[0] // 4
                        um = pair[0] % 4
                        if lo_min < fhi_max:
                            nc.vector.tensor_add(
                                pst[:, : len(pair), lo_min:fhi_max],
                                pst[:, : len(pair), lo_min:fhi_max],
                                mask_t[g][:, um : um + len(pair), lo_min - mlo[g] : fhi_max - mlo[g]],
                            )
                        e = esb.tile([128, 2, M], BF16, name="e", tag="e")
                        nc.scalar.activation(
                            e[:, : len(pair), lo_min:M],
                            pst[:, : len(pair), lo_min:M],
                            mybir.ActivationFunctionType.Exp,
                            bias=zbias[:],
                        )
                        for u, i in enumerate(pair):
                            cur.append((i, e, u, flo[i]))
                    if pend:
                        for i, e, u, lo in pend:
                            nc.tensor.matmul(
                                po[:, lo:M],
                                v_t[i][:, j * 128 : (j + 1) * 128],
                                e[:, u, lo:M],
                                start=first, stop=False, skip_group_check=True,
                            )
                            first = False
                        for i, e, u, lo in pend:
                            nc.tensor.matmul(
                                pl[:, lo:M], ones_c[:], e[:, u, lo:M],
                                start=lfirst, stop=False, skip_group_check=True,
                            )
                            lfirst = False
                    pend = cur
                    if bi == 1:
                        # prefetch next head's K^T; emit previous head's norm
                        if j + 1 < NHG:
                            kth[j + 1] = kthp.tile([128, T], BF16, name="kth", tag="kth")
                            nc.sync.dma_start(kth[j + 1][:], ktd[j + 1])
                        if j > 0 and (j - 1) in po_q:
                            emit_norm(j - 1)
                for k, (i, e, u, lo) in enumerate(pend):
                    last = k == len(pend) - 1
                    nc.tensor.matmul(
                        po[:, lo:M],
                        v_t[i][:, j * 128 : (j + 1) * 128],
                        e[:, u, lo:M],
                        start=first, stop=last, skip_group_check=True,
                    )
                    first = False
                for k, (i, e, u, lo) in enumerate(pend):
                    nc.tensor.matmul(
                        pl[:, lo:M], ones_c[:], e[:, u, lo:M],
                        start=lfirst, stop=(k == len(pend) - 1),
                        skip_group_check=True,
                    )
                    lfirst = False
                kth.pop(j)
                if j > 0 and (j - 1) in po_q:
                    emit_norm(j - 1)
            emit_norm(NHG - 1)

        # ---- phase C: y = O @ wo  (row-parallel partial) -----------
        with (
            nc.named_scope("phase_C"),
            tc.tile_pool(name="py", bufs=2, space="PSUM") as pyp,
            tc.tile_pool(name="ysb", bufs=3) as ysb,
        ):
            # fo pairs share the stationary ot slice -> one weight load
            # feeds two 512-wide matmuls
            for mb in range(M // 128):
                for fp in range(D // 1024):
                    py = [
                        pyp.tile([128, 512], F32, name="py", tag=f"py{h}")
                        for h in range(2)
                    ]
                    for j in range(NHG):
                        for h in range(2):
                            fo = 2 * fp + h
                            nc.tensor.matmul(
                                py[h][:],
                                ot_t[j][:, mb * 128 : (mb + 1) * 128],
                                wo_t[j // 4][:, j % 4, fo * 512 : (fo + 1) * 512],
                                start=(j == 0),
                                stop=(j == NHG - 1),
                                skip_group_check=True,
                            )
                    for h in range(2):
                        ys = ysb.tile([128, 512], F32, name="ys", tag="ys")
                        nc.scalar.copy(ys[:], py[h][:])
                        nc.sync.dma_start(
                            y[
                                mb * 128 : (mb + 1) * 128,
                                (2 * fp + h) * 512 : (2 * fp + h + 1) * 512,
                            ],
                            ys[:],
                        )

    nc.compile()
    return nc


_cache = {}


def _get_program(flo, fhi):
    key = (tuple(flo), tuple(fhi))
    if key not in _cache:
        _cache[key] = build_program(list(flo), list(fhi))
    return _cache[key]


def _prep(inputs):
    x = np.asarray(inputs["x"], dtype=np.float32)
    qidx = np.asarray(inputs["query_idx"]).astype(np.int64)
    Wq = np.asarray(inputs["Wq"], dtype=np.float32)
    Wk = np.asarray(inputs["Wk"], dtype=np.float32)
    Wv = np.asarray(inputs["Wv"], dtype=np.float32)
    Wo = np.asarray(inputs["Wo"], dtype=np.float32)
    bq = np.asarray(inputs["bq"], dtype=np.float32)
    bk = np.asarray(inputs["bk"], dtype=np.float32)
    bv = np.asarray(inputs["bv"], dtype=np.float32)
    bo = np.asarray(inputs["bo"], dtype=np.float32)

    # Per-t-chunk skip bounds, union over batches.  flo[i] = first m that
    # attends into chunk i (everything below is fully masked there);
    # fhi[i] = one past the last m only partially covered by chunk i.
    # Computed positionally so they are correct even for unsorted
    # query_idx (just less effective at skipping).
    flo = [M] * NT
    fhi = [0] * NT
    for b in range(B):
        for i in range(NT):
            allowed = qidx[b] >= 128 * i          # chunk i not fully masked
            partial = qidx[b] < 128 * (i + 1)     # chunk i not fully allowed
            lo_b = int(np.argmax(allowed)) if allowed.any() else M
            hi_b = M - int(np.argmax(partial[::-1])) if partial.any() else 0
            flo[i] = min(flo[i], lo_b)
            fhi[i] = max(fhi[i], hi_b)

    in_maps = []
    tgrid = np.arange(T)[:, None]
    for core in range(8):
        b, g = divmod(core, 2)
        sl = slice(g * DG, (g + 1) * DG)
        xb = x[b]
        mask = np.where(tgrid <= qidx[b][None, :], np.float32(0), MASK_VAL)
        in_maps.append(
            {
                "xT": np.ascontiguousarray(xb.T.astype(NPBF)),
                "xqT": np.ascontiguousarray(xb[qidx[b]].T.astype(NPBF)),
                "wk": np.ascontiguousarray(Wk[:, sl].astype(NPBF)),
                "wv": np.ascontiguousarray(Wv[:, sl].astype(NPBF)),
                "wq": np.ascontiguousarray(Wq[:, sl].astype(NPBF)),
                "wo": np.ascontiguousarray(Wo[sl, :].astype(NPBF)),
                "mask": np.ascontiguousarray(mask.astype(NPBF)),
                "bks": np.ascontiguousarray(bk[sl].reshape(NHG, 128).T),
                "bqs": np.ascontiguousarray(
                    (bq[sl] / np.sqrt(HD)).reshape(NHG, 128).T.astype(np.float32)
                ),
            }
        )

    const = (bv.astype(np.float64) @ Wo.astype(np.float64) + bo).astype(np.float32)
    return flo, fhi, in_maps, const


def run(inputs, trace=False, trace_kwargs=None):
    _install_ntff_hook()
    flo, fhi, in_maps, const = _prep(inputs)
    nc = _get_program(flo, fhi)
    res = run_bass_kernel_spmd(
        nc, in_maps, list(range(8)), trace=trace, **(trace_kwargs or {})
    )
    out = np.zeros((B, M, D), dtype=np.float32)
    for b in range(B):
        out[b] = res.results[2 * b]["y"] + res.results[2 * b + 1]["y"] + const
    return out, res


def kernel(**inputs) -> np.ndarray:
    out, _ = run(inputs, trace=False)
    return out


# revision 25
# speedup vs baseline: 1.0053x; 1.0053x over previous
"""Trainium2 Bass kernel for sparse causal attention (nn_CausalAttentionKV).

Reference computation (fp32, single device):
    q_all = x @ Wq + bq ; k_all = x @ Wk + bk ; v_all = x @ Wv + bv
    q = gather(q_all, query_idx)        # (B, M, D) selected query rows
    att = softmax(mask(q k^T / sqrt(hd)))   # per-query causal mask t <= qidx[m]
    y = (att v) @ Wo + bo

Shapes: B=4, T=4096, D=2048, n_head=16, hd=128, M=512.

Sharding (8 cores): core = 2*b + g  handles batch b and head-group g
(8 heads = 1024 feature cols).  Q/K/V projections are column-parallel,
out-proj is row-parallel; the two partial outputs per batch are summed
on the host.  All matmul inputs are bf16 (fp32 PSUM accumulation).

Host-side prep per core: transpose x/xq to (D, T) layout (the PE needs
the contraction dim on partitions), gather the M query rows of x,
slice/cast weights, precompute the additive causal mask, and compute
per-t-block skip bounds from the (sorted) query_idx so fully-masked
regions of the score matrix are never computed.

Per-core schedule: Q projection first (small, covers the K/V weight
prefetch), then one fused pass over x computing K^T (streamed out to a
DRAM scratch) and V (resident in SBUF), then per-head attention with
K^T streamed back in, then the output projection.
"""

import sys
import types
from contextlib import ExitStack

import numpy as np
import ml_dtypes

import concourse.bass as bass
import concourse.tile as tile
import concourse.mybir as mybir
from concourse import bacc
from concourse.bass_utils import run_bass_kernel_spmd

BF16 = mybir.dt.bfloat16
F32 = mybir.dt.float32
NPBF = ml_dtypes.bfloat16

B, T, D = 4, 4096, 2048
NH, HD, M = 16, 128, 512
NHG = 8            # heads per core (group)
DG = NHG * HD      # 1024 feature cols per core
NT = T // 128      # 32 t-chunks
ND = D // 128      # 16 d-chunks
MASK_VAL = np.float32(-30000.0)


def _install_ntff_hook():
    """Register the axon NTFF profiling hook if the image's antenv lacks it."""
    try:
        from antenv.axon_hooks import get_axon_ntff_profile_hook  # noqa: F401
        return
    except ImportError:
        pass
    try:
        import antenv
        from trn_agent_boot.trn_boot import _ntff_profile_via_ctypes

        mod = types.ModuleType("antenv.axon_hooks")
        hook = [None]
        mod.set_axon_ntff_profile_hook = lambda h: hook.__setitem__(0, h)
        mod.get_axon_ntff_profile_hook = lambda: hook[0]
        sys.modules["antenv.axon_hooks"] = mod
        antenv.axon_hooks = mod
        mod.set_axon_ntff_profile_hook(
            _ntff_profile_via_ctypes("/opt/axon/libaxon_pjrt.so")
        )
    except Exception:
        pass


def build_program(flo, fhi):
    """Build the per-core Bass program.

    flo[i]: first m column with any allowed key in t-chunk i (cols below
            are fully masked there -> never computed).
    fhi[i]: first m column fully allowed in t-chunk i (cols beyond need
            no mask add).
    Both are unions over the 4 batches so one program serves all cores.
    """
    nc = bacc.Bacc("TRN2", target_bir_lowering=False, debug=False)

    xT = nc.dram_tensor("xT", [D, T], BF16, kind="ExternalInput")
    xqT = nc.dram_tensor("xqT", [D, M], BF16, kind="ExternalInput")
    wk = nc.dram_tensor("wk", [D, DG], BF16, kind="ExternalInput")
    wv = nc.dram_tensor("wv", [D, DG], BF16, kind="ExternalInput")
    wq = nc.dram_tensor("wq", [D, DG], BF16, kind="ExternalInput")
    wo = nc.dram_tensor("wo", [DG, D], BF16, kind="ExternalInput")
    maskd = nc.dram_tensor("mask", [T, M], BF16, kind="ExternalInput")
    bks = nc.dram_tensor("bks", [128, NHG], F32, kind="ExternalInput")
    bqs = nc.dram_tensor("bqs", [128, NHG], F32, kind="ExternalInput")
    y = nc.dram_tensor("y", [M, D], F32, kind="ExternalOutput")

    # (c*128+p, t) views for 4-chunk batched DMA
    xTr = xT.rearrange("(c p) t -> p c t", p=128)
    xqTr = xqT.rearrange("(c p) t -> p c t", p=128)
    wkr = wk.rearrange("(c p) t -> p c t", p=128)
    wvr = wv.rearrange("(c p) t -> p c t", p=128)
    wqr = wq.rearrange("(c p) t -> p c t", p=128)
    wor = wo.rearrange("(c p) t -> p c t", p=128)
    maskr = maskd.rearrange("(c p) t -> p c t", p=128)

    with ExitStack() as ctx:
        tc = ctx.enter_context(tile.TileContext(nc))

        # ---- persistent tiles --------------------------------------
        persist = ctx.enter_context(tc.tile_pool(name="persist", bufs=1))
        v_t = [persist.tile([128, DG], BF16, name=f"v{i}", tag=f"v{i}") for i in range(NT)]
        qt_t = [persist.tile([128, M], BF16, name=f"qt{j}", tag=f"qt{j}") for j in range(NHG)]
        ot_t = [persist.tile([128, M], BF16, name=f"ot{j}", tag=f"ot{j}") for j in range(NHG)]
        bias_k = persist.tile([128, NHG], F32, name="bias_k", tag="bias_k")
        bias_q = persist.tile([128, NHG], F32, name="bias_q", tag="bias_q")
        zbias = persist.tile([128, 1], F32, name="zbias", tag="zbias")
        ones_c = persist.tile([128, 1], BF16, name="ones_c", tag="ones_c")
        ones_r = persist.tile([1, 128], F32, name="ones_r", tag="ones_r")
        # mask super-tiles: 4 t-chunks each, shared col-window
        mlo = [min(flo[4 * g : 4 * g + 4]) for g in range(NT // 4)]
        mhi = [max(fhi[4 * g : 4 * g + 4]) for g in range(NT // 4)]
        mask_t = [
            persist.tile(
                [128, 4, max(mhi[g] - mlo[g], 1)], BF16,
                name=f"mask{g}", tag=f"mask{g}",
            )
            for g in range(NT // 4)
        ]
        dram = ctx.enter_context(tc.tile_pool(name="dram", bufs=1, space="DRAM"))
        ktd = dram.tile([NHG, 128, T], BF16, name="ktd")
        ktd_r = ktd.rearrange("j p t -> p j t")

        nc.sync.dma_start(bias_k[:], bks[:])
        nc.sync.dma_start(bias_q[:], bqs[:])
        nc.vector.memset(zbias[:], 0.0)
        nc.vector.memset(ones_c[:], 1.0)
        nc.vector.memset(ones_r[:], 1.0)

        # ---- phase A-Q: Qt[j] = ((xq @ wq_j + bq_j)/sqrt(hd))^T ----
        # wq is loaded in per-head column slices so the first matmul
        # group only waits for ~0.5 MB; runs while wk/wv prefetch.
        with (
            nc.named_scope("phase_AQ"),
            tc.tile_pool(name="wqp", bufs=1) as wqp,
            tc.tile_pool(name="xqp", bufs=1) as xqp,
            tc.tile_pool(name="pq", bufs=4, space="PSUM") as pqp,
        ):
            xq_t = [xqp.tile([128, 4, M], BF16, name=f"xq{d}", tag=f"xq{d}") for d in range(4)]
            for d in range(4):
                nc.sync.dma_start(xq_t[d][:], xqTr[:, 4 * d : 4 * d + 4, :])
            # wq in head-pair column slices: (jg, s) -> 4 d-chunks x 256 cols
            wq_t = {}
            for jg in range(4):
                for s in range(4):
                    wq_t[jg, s] = wqp.tile(
                        [128, 4, 256], BF16, name=f"wq{jg}_{s}", tag=f"wq{jg}_{s}"
                    )
                    nc.sync.dma_start(
                        wq_t[jg, s][:],
                        wqr[:, 4 * s : 4 * s + 4, jg * 256 : (jg + 1) * 256],
                    )
            inv_s = 1.0 / float(np.sqrt(HD))
            for j in range(NHG):
                jg, co = j // 2, (j % 2) * 128
                pq = pqp.tile([128, M], F32, name="pq", tag="pq")
                for d in range(ND):
                    nc.tensor.matmul(
                        pq[:],
                        wq_t[jg, d // 4][:, d % 4, co : co + 128],
                        xq_t[d // 4][:, d % 4, :],
                        start=(d == 0),
                        stop=(d == ND - 1),
                    )
                nc.scalar.activation(
                    qt_t[j][:],
                    pq[:],
                    mybir.ActivationFunctionType.Identity,
                    scale=inv_s,
                    bias=bias_q[:, j : j + 1],
                )

        # ---- phase A-KV: one pass over x computing Kt and V --------
        KTS = 512
        with (
            nc.named_scope("phase_AKV"),
            tc.tile_pool(name="wkp", bufs=1) as wkp,
            tc.tile_pool(name="wvp", bufs=1) as wvp,
            tc.tile_pool(name="xtp", bufs=2) as xtp,
            tc.tile_pool(name="kst", bufs=3) as kstp,
            tc.tile_pool(name="pk", bufs=3, space="PSUM") as pkp,
            tc.tile_pool(name="pv", bufs=3, space="PSUM") as pvp,
        ):
            wk_t = [wkp.tile([128, 4, DG], BF16, name=f"wk{d}", tag=f"wk{d}") for d in range(4)]
            wv_t = [wvp.tile([128, 4, DG], BF16, name=f"wv{d}", tag=f"wv{d}") for d in range(4)]
            for d in range(4):
                nc.sync.dma_start(wk_t[d][:], wkr[:, 4 * d : 4 * d + 4, :])
            for d in range(4):
                nc.sync.dma_start(wv_t[d][:], wvr[:, 4 * d : 4 * d + 4, :])
            for ts in range(T // KTS):
                xt_t = [xtp.tile([128, 4, KTS], BF16, name=f"xt{d}", tag=f"xt{d}") for d in range(4)]
                for d in range(4):
                    nc.sync.dma_start(
                        xt_t[d][:], xTr[:, 4 * d : 4 * d + 4, ts * KTS : (ts + 1) * KTS]
                    )
                # K^T: per head j, (hd, KTS) tile; staged 4 heads per DMA
                for jg in range(2):
                    ks = kstp.tile([128, 4, KTS], BF16, name="ks", tag="ks")
                    for jj in range(4):
                        j = 4 * jg + jj
                        pk = pkp.tile([128, KTS], F32, name="pk", tag="pk")
                        for d in range(ND):
                            nc.tensor.matmul(
                                pk[:],
                                wk_t[d // 4][:, d % 4, j * 128 : (j + 1) * 128],
                                xt_t[d // 4][:, d % 4, :],
                                start=(d == 0),
                                stop=(d == ND - 1),
                            )
                        nc.scalar.activation(
                            ks[:, jj, :],
                            pk[:],
                            mybir.ActivationFunctionType.Identity,
                            bias=bias_k[:, j : j + 1],
                        )
                    nc.sync.dma_start(
                        ktd_r[:, 4 * jg : 4 * jg + 4, ts * KTS : (ts + 1) * KTS],
                        ks[:],
                    )
                # V: (t, DG) tiles
                for u in range(KTS // 128):
                    i = ts * (KTS // 128) + u
                    for f in range(2):
                        pv = pvp.tile([128, 512], F32, name="pv", tag="pv")
                        for d in range(ND):
                            nc.tensor.matmul(
                                pv[:],
                                xt_t[d // 4][:, d % 4, u * 128 : (u + 1) * 128],
                                wv_t[d // 4][:, d % 4, f * 512 : (f + 1) * 512],
                                start=(d == 0),
                                stop=(d == ND - 1),
                            )
                        nc.vector.tensor_copy(
                            v_t[i][:, f * 512 : (f + 1) * 512], pv[:]
                        )

        # ---- phase B prefetch: out-proj weights + masks ------------
        wop = ctx.enter_context(tc.tile_pool(name="wop", bufs=1))
        wo_t = [wop.tile([128, 4, D], BF16, name=f"wo{d}", tag=f"wo{d}") for d in range(2)]
        for d in range(2):
            nc.sync.dma_start(wo_t[d][:], wor[:, 4 * d : 4 * d + 4, :])
        for g in range(NT // 4):
            if mlo[g] < M and mhi[g] > mlo[g]:
                nc.sync.dma_start(
                    mask_t[g][:, :, : mhi[g] - mlo[g]],
                    maskr[:, 4 * g : 4 * g + 4, mlo[g] : mhi[g]],
                )

        # ---- phase B: attention per head, 4-chunk batched ----------
        chunks = [i for i in range(NT) if flo[i] < M]
        pairs = [chunks[k : k + 2] for k in range(0, len(chunks), 2)]
        batches = [pairs[k : k + 2] for k in range(0, len(pairs), 2)]
        with (
            nc.named_scope("phase_B"),
            tc.tile_pool(name="kth", bufs=2) as kthp,
            tc.tile_pool(name="ps", bufs=2, space="PSUM") as psp,
            tc.tile_pool(name="po", bufs=2, space="PSUM") as pop,
            tc.tile_pool(name="pl", bufs=2, space="PSUM") as plp,
            tc.tile_pool(name="esb", bufs=5) as esb,
            tc.tile_pool(name="lsb", bufs=2) as lsb,
        ):
            po_q, pl_q = {}, {}

            def emit_norm(j):
                """Normalize head j: ot[j] = po[j] / l[j] (off PE critical path).

                l is broadcast across partitions on the PE first, so the
                reciprocal runs 128 lanes wide instead of on 1 partition.
                """
                po, pl = po_q.pop(j), pl_q.pop(j)
                l_sb = lsb.tile([1, M], F32, name="l", tag="l")
                linv = lsb.tile([1, M], F32, name="linv", tag="linv")
                nc.vector.tensor_copy(l_sb[:], pl[:])
                nc.vector.reciprocal(linv[:], l_sb[:])
                pb = psp.tile([128, M], F32, name="pb", tag="ps")
                nc.tensor.matmul(pb[:], ones_r[:], linv[:], start=True, stop=True)
                lb_sb = lsb.tile([128, M], F32, name="lb", tag="lb")
                nc.scalar.copy(lb_sb[:], pb[:])
                nc.vector.tensor_mul(ot_t[j][:], po[:], lb_sb[:])

            kth = {}
            for j in range(NHG):
                if j == 0:
                    kth[0] = kthp.tile([128, T], BF16, name="kth", tag="kth")
                    nc.sync.dma_start(kth[0][:], ktd[0])
                po = pop.tile([128, M], F32, name="po", tag="po")
                pl = plp.tile([1, M], F32, name="pl", tag="pl")
                po_q[j], pl_q[j] = po, pl
                pend = None  # previous batch: list of (i, e, u, lo)
                first = True
                lfirst = True
                for bi, batch in enumerate(batches):
                    cur = []
                    for pair in batch:
                        lo_min = min(flo[i] for i in pair)
                        pst = psp.tile([128, 2, M], F32, name="pst", tag="ps")
                        for u, i in enumerate(pair):
                            nc.tensor.matmul(
                                pst[:, u, flo[i] : M],
                                kth[j][:, i * 128 : (i + 1) * 128],
                                qt_t[j][:, flo[i] : M],
                                start=True,
                                stop=True,
                                skip_group_check=True,
                            )
                        # one mask add + one exp per pair; mask cols beyond a
                        # chunk's own [lo, fhi) window add 0 or touch lanes
                        # the narrower chunk never reads
                        fhi_max = max(fhi[i] for i in pair)
                        g = pair[0] // 4
                        um = pair[0] % 4
                        if lo_min < fhi_max:
                            nc.vector.tensor_add(
                                pst[:, : len(pair), lo_min:fhi_max],
                                pst[:, : len(pair), lo_min:fhi_max],
                                mask_t[g][:, um : um + len(pair), lo_min - mlo[g] : fhi_max - mlo[g]],
                            )
                        e = esb.tile([128, 2, M], BF16, name="e", tag="e")
                        nc.scalar.activation(
                            e[:, : len(pair), lo_min:M],
                            pst[:, : len(pair), lo_min:M],
                            mybir.ActivationFunctionType.Exp,
                            bias=zbias[:],
                        )
                        for u, i in enumerate(pair):
                            cur.append((i, e, u, flo[i]))
                    if pend:
                        for i, e, u, lo in pend:
                            nc.tensor.matmul(
                                po[:, lo:M],
                                v_t[i][:, j * 128 : (j + 1) * 128],
                                e[:, u, lo:M],
                                start=first, stop=False, skip_group_check=True,
                            )
                            first = False
                        for i, e, u, lo in pend:
                            nc.tensor.matmul(
                                pl[:, lo:M], ones_c[:], e[:, u, lo:M],
                                start=lfirst, stop=False, skip_group_check=True,
                            )
                            lfirst = False
                    pend = cur
                    if bi == 2:
                        # prefetch next head's K^T; emit previous head's norm
                        if j + 1 < NHG:
                            kth[j + 1] = kthp.tile([128, T], BF16, name="kth", tag="kth")
                            nc.sync.dma_start(kth[j + 1][:], ktd[j + 1])
                        if j > 0 and (j - 1) in po_q:
                            emit_norm(j - 1)
                for k, (i, e, u, lo) in enumerate(pend):
                    last = k == len(pend) - 1
                    nc.tensor.matmul(
                        po[:, lo:M],
                        v_t[i][:, j * 128 : (j + 1) * 128],
                        e[:, u, lo:M],
                        start=first, stop=last, skip_group_check=True,
                    )
                    first = False
                for k, (i, e, u, lo) in enumerate(pend):
                    nc.tensor.matmul(
                        pl[:, lo:M], ones_c[:], e[:, u, lo:M],
                        start=lfirst, stop=(k == len(pend) - 1),
                        skip_group_check=True,
                    )
                    lfirst = False
                kth.pop(j)
                if j > 0 and (j - 1) in po_q:
                    emit_norm(j - 1)
            emit_norm(NHG - 1)

        # ---- phase C: y = O @ wo  (row-parallel partial) -----------
        with (
            nc.named_scope("phase_C"),
            tc.tile_pool(name="py", bufs=2, space="PSUM") as pyp,
            tc.tile_pool(name="ysb", bufs=3) as ysb,
        ):
            # fo pairs share the stationary ot slice -> one weight load
            # feeds two 512-wide matmuls
            for mb in range(M // 128):
                for fp in range(D // 1024):
                    py = [
                        pyp.tile([128, 512], F32, name="py", tag=f"py{h}")
                        for h in range(2)
                    ]
                    for j in range(NHG):
                        for h in range(2):
                            fo = 2 * fp + h
                            nc.tensor.matmul(
                                py[h][:],
                                ot_t[j][:, mb * 128 : (mb + 1) * 128],
                                wo_t[j // 4][:, j % 4, fo * 512 : (fo + 1) * 512],
                                start=(j == 0),
                                stop=(j == NHG - 1),
                                skip_group_check=True,
                            )
                    for h in range(2):
                        ys = ysb.tile([128, 512], F32, name="ys", tag="ys")
                        nc.scalar.copy(ys[:], py[h][:])
                        nc.sync.dma_start(
                            y[
                                mb * 128 : (mb + 1) * 128,
                                (2 * fp + h) * 512 : (2 * fp + h + 1) * 512,
                            ],
                            ys[:],
                        )

    nc.compile()
    return nc


_cache = {}


def _get_program(flo, fhi):
    key = (tuple(flo), tuple(fhi))
    if key not in _cache:
        _cache[key] = build_program(list(flo), list(fhi))
    return _cache[key]


def _prep(inputs):
    x = np.asarray(inputs["x"], dtype=np.float32)
    qidx = np.asarray(inputs["query_idx"]).astype(np.int64)
    Wq = np.asarray(inputs["Wq"], dtype=np.float32)
    Wk = np.asarray(inputs["Wk"], dtype=np.float32)
    Wv = np.asarray(inputs["Wv"], dtype=np.float32)
    Wo = np.asarray(inputs["Wo"], dtype=np.float32)
    bq = np.asarray(inputs["bq"], dtype=np.float32)
    bk = np.asarray(inputs["bk"], dtype=np.float32)
    bv = np.asarray(inputs["bv"], dtype=np.float32)
    bo = np.asarray(inputs["bo"], dtype=np.float32)

    # Per-t-chunk skip bounds, union over batches.  flo[i] = first m that
    # attends into chunk i (everything below is fully masked there);
    # fhi[i] = one past the last m only partially covered by chunk i.
    # Computed positionally so they are correct even for unsorted
    # query_idx (just less effective at skipping).
    flo = [M] * NT
    fhi = [0] * NT
    for b in range(B):
        for i in range(NT):
            allowed = qidx[b] >= 128 * i          # chunk i not fully masked
            partial = qidx[b] < 128 * (i + 1)     # chunk i not fully allowed
            lo_b = int(np.argmax(allowed)) if allowed.any() else M
            hi_b = M - int(np.argmax(partial[::-1])) if partial.any() else 0
            flo[i] = min(flo[i], lo_b)
            fhi[i] = max(fhi[i], hi_b)

    in_maps = []
    tgrid = np.arange(T)[:, None]
    for core in range(8):
        b, g = divmod(core, 2)
        sl = slice(g * DG, (g + 1) * DG)
        xb = x[b]
        mask = np.where(tgrid <= qidx[b][None, :], np.float32(0), MASK_VAL)
        in_maps.append(
            {
                "xT": np.ascontiguousarray(xb.T.astype(NPBF)),
                "xqT": np.ascontiguousarray(xb[qidx[b]].T.astype(NPBF)),
                "wk": np.ascontiguousarray(Wk[:, sl].astype(NPBF)),
                "wv": np.ascontiguousarray(Wv[:, sl].astype(NPBF)),
                "wq": np.ascontiguousarray(Wq[:, sl].astype(NPBF)),
                "wo": np.ascontiguousarray(Wo[sl, :].astype(NPBF)),
                "mask": np.ascontiguousarray(mask.astype(NPBF)),
                "bks": np.ascontiguousarray(bk[sl].reshape(NHG, 128).T),
                "bqs": np.ascontiguousarray(
                    (bq[sl] / np.sqrt(HD)).reshape(NHG, 128).T.astype(np.float32)
                ),
            }
        )

    const = (bv.astype(np.float64) @ Wo.astype(np.float64) + bo).astype(np.float32)
    return flo, fhi, in_maps, const


def run(inputs, trace=False, trace_kwargs=None):
    _install_ntff_hook()
    flo, fhi, in_maps, const = _prep(inputs)
    nc = _get_program(flo, fhi)
    res = run_bass_kernel_spmd(
        nc, in_maps, list(range(8)), trace=trace, **(trace_kwargs or {})
    )
    out = np.zeros((B, M, D), dtype=np.float32)
    for b in range(B):
        out[b] = res.results[2 * b]["y"] + res.results[2 * b + 1]["y"] + const
    return out, res


def kernel(**inputs) -> np.ndarray:
    out, _ = run(inputs, trace=False)
    return out


# revision 26
# speedup vs baseline: 1.0223x; 1.0168x over previous
"""Trainium2 Bass kernel for sparse causal attention (nn_CausalAttentionKV).

Reference computation (fp32, single device):
    q_all = x @ Wq + bq ; k_all = x @ Wk + bk ; v_all = x @ Wv + bv
    q = gather(q_all, query_idx)        # (B, M, D) selected query rows
    att = softmax(mask(q k^T / sqrt(hd)))   # per-query causal mask t <= qidx[m]
    y = (att v) @ Wo + bo

Shapes: B=4, T=4096, D=2048, n_head=16, hd=128, M=512.

Sharding (8 cores): core = 2*b + g  handles batch b and head-group g
(8 heads = 1024 feature cols).  Q/K/V projections are column-parallel,
out-proj is row-parallel; the two partial outputs per batch are summed
on the host.  All matmul inputs are bf16 (fp32 PSUM accumulation).

Host-side prep per core: transpose x/xq to (D, T) layout (the PE needs
the contraction dim on partitions), gather the M query rows of x,
slice/cast weights, precompute the additive causal mask, and compute
per-t-block skip bounds from the (sorted) query_idx so fully-masked
regions of the score matrix are never computed.

Per-core schedule: Q projection first (small, covers the K/V weight
prefetch), then one fused pass over x computing K^T (streamed out to a
DRAM scratch) and V (resident in SBUF), then per-head attention with
K^T streamed back in, then the output projection.
"""

import sys
import types
from contextlib import ExitStack

import numpy as np
import ml_dtypes

import concourse.bass as bass
import concourse.tile as tile
import concourse.mybir as mybir
from concourse import bacc
from concourse.bass_utils import run_bass_kernel_spmd

BF16 = mybir.dt.bfloat16
F32 = mybir.dt.float32
NPBF = ml_dtypes.bfloat16

B, T, D = 4, 4096, 2048
NH, HD, M = 16, 128, 512
NHG = 8            # heads per core (group)
DG = NHG * HD      # 1024 feature cols per core
NT = T // 128      # 32 t-chunks
ND = D // 128      # 16 d-chunks
MASK_VAL = np.float32(-30000.0)


def _install_ntff_hook():
    """Register the axon NTFF profiling hook if the image's antenv lacks it."""
    try:
        from antenv.axon_hooks import get_axon_ntff_profile_hook  # noqa: F401
        return
    except ImportError:
        pass
    try:
        import antenv
        from trn_agent_boot.trn_boot import _ntff_profile_via_ctypes

        mod = types.ModuleType("antenv.axon_hooks")
        hook = [None]
        mod.set_axon_ntff_profile_hook = lambda h: hook.__setitem__(0, h)
        mod.get_axon_ntff_profile_hook = lambda: hook[0]
        sys.modules["antenv.axon_hooks"] = mod
        antenv.axon_hooks = mod
        mod.set_axon_ntff_profile_hook(
            _ntff_profile_via_ctypes("/opt/axon/libaxon_pjrt.so")
        )
    except Exception:
        pass


def build_program(flo, fhi):
    """Build the per-core Bass program.

    flo[i]: first m column with any allowed key in t-chunk i (cols below
            are fully masked there -> never computed).
    fhi[i]: first m column fully allowed in t-chunk i (cols beyond need
            no mask add).
    Both are unions over the 4 batches so one program serves all cores.
    """
    nc = bacc.Bacc("TRN2", target_bir_lowering=False, debug=False)

    xT = nc.dram_tensor("xT", [D, T], BF16, kind="ExternalInput")
    xqT = nc.dram_tensor("xqT", [D, M], BF16, kind="ExternalInput")
    wk = nc.dram_tensor("wk", [D, DG], BF16, kind="ExternalInput")
    wv = nc.dram_tensor("wv", [D, DG], BF16, kind="ExternalInput")
    wq = nc.dram_tensor("wq", [D, DG], BF16, kind="ExternalInput")
    wo = nc.dram_tensor("wo", [DG, D], BF16, kind="ExternalInput")
    maskd = nc.dram_tensor("mask", [T, M], BF16, kind="ExternalInput")
    bks = nc.dram_tensor("bks", [128, NHG], F32, kind="ExternalInput")
    bqs = nc.dram_tensor("bqs", [128, NHG], F32, kind="ExternalInput")
    y = nc.dram_tensor("y", [M, D], F32, kind="ExternalOutput")

    # (c*128+p, t) views for 4-chunk batched DMA
    xTr = xT.rearrange("(c p) t -> p c t", p=128)
    xqTr = xqT.rearrange("(c p) t -> p c t", p=128)
    wkr = wk.rearrange("(c p) t -> p c t", p=128)
    wvr = wv.rearrange("(c p) t -> p c t", p=128)
    wqr = wq.rearrange("(c p) t -> p c t", p=128)
    wor = wo.rearrange("(c p) t -> p c t", p=128)
    maskr = maskd.rearrange("(c p) t -> p c t", p=128)

    with ExitStack() as ctx:
        tc = ctx.enter_context(tile.TileContext(nc))

        # ---- persistent tiles --------------------------------------
        persist = ctx.enter_context(tc.tile_pool(name="persist", bufs=1))
        v_t = [persist.tile([128, DG], BF16, name=f"v{i}", tag=f"v{i}") for i in range(NT)]
        qt_t = [persist.tile([128, M], BF16, name=f"qt{j}", tag=f"qt{j}") for j in range(NHG)]
        ot_t = [persist.tile([128, M], BF16, name=f"ot{j}", tag=f"ot{j}") for j in range(NHG)]
        bias_k = persist.tile([128, NHG], F32, name="bias_k", tag="bias_k")
        bias_q = persist.tile([128, NHG], F32, name="bias_q", tag="bias_q")
        zbias = persist.tile([128, 1], F32, name="zbias", tag="zbias")
        ones_c = persist.tile([128, 1], BF16, name="ones_c", tag="ones_c")
        ones_r = persist.tile([1, 128], F32, name="ones_r", tag="ones_r")
        kt0_sb = persist.tile([128, T], BF16, name="kt0_sb", tag="kt0_sb")
        # mask super-tiles: 4 t-chunks each, shared col-window
        mlo = [min(flo[4 * g : 4 * g + 4]) for g in range(NT // 4)]
        mhi = [max(fhi[4 * g : 4 * g + 4]) for g in range(NT // 4)]
        mask_t = [
            persist.tile(
                [128, 4, max(mhi[g] - mlo[g], 1)], BF16,
                name=f"mask{g}", tag=f"mask{g}",
            )
            for g in range(NT // 4)
        ]
        dram = ctx.enter_context(tc.tile_pool(name="dram", bufs=1, space="DRAM"))
        ktd = dram.tile([NHG, 128, T], BF16, name="ktd")
        ktd_r = ktd.rearrange("j p t -> p j t")

        nc.sync.dma_start(bias_k[:], bks[:])
        nc.sync.dma_start(bias_q[:], bqs[:])
        nc.vector.memset(zbias[:], 0.0)
        nc.vector.memset(ones_c[:], 1.0)
        nc.vector.memset(ones_r[:], 1.0)

        # ---- phase A-Q: Qt[j] = ((xq @ wq_j + bq_j)/sqrt(hd))^T ----
        # wq is loaded in per-head column slices so the first matmul
        # group only waits for ~0.5 MB; runs while wk/wv prefetch.
        with (
            nc.named_scope("phase_AQ"),
            tc.tile_pool(name="wqp", bufs=1) as wqp,
            tc.tile_pool(name="xqp", bufs=1) as xqp,
            tc.tile_pool(name="pq", bufs=4, space="PSUM") as pqp,
        ):
            xq_t = [xqp.tile([128, 4, M], BF16, name=f"xq{d}", tag=f"xq{d}") for d in range(4)]
            for d in range(4):
                nc.sync.dma_start(xq_t[d][:], xqTr[:, 4 * d : 4 * d + 4, :])
            # wq in head-pair column slices: (jg, s) -> 4 d-chunks x 256 cols
            wq_t = {}
            for jg in range(4):
                for s in range(4):
                    wq_t[jg, s] = wqp.tile(
                        [128, 4, 256], BF16, name=f"wq{jg}_{s}", tag=f"wq{jg}_{s}"
                    )
                    nc.sync.dma_start(
                        wq_t[jg, s][:],
                        wqr[:, 4 * s : 4 * s + 4, jg * 256 : (jg + 1) * 256],
                    )
            inv_s = 1.0 / float(np.sqrt(HD))
            for j in range(NHG):
                jg, co = j // 2, (j % 2) * 128
                pq = pqp.tile([128, M], F32, name="pq", tag="pq")
                for d in range(ND):
                    nc.tensor.matmul(
                        pq[:],
                        wq_t[jg, d // 4][:, d % 4, co : co + 128],
                        xq_t[d // 4][:, d % 4, :],
                        start=(d == 0),
                        stop=(d == ND - 1),
                    )
                nc.scalar.activation(
                    qt_t[j][:],
                    pq[:],
                    mybir.ActivationFunctionType.Identity,
                    scale=inv_s,
                    bias=bias_q[:, j : j + 1],
                )

        # ---- phase A-KV: one pass over x computing Kt and V --------
        KTS = 512
        with (
            nc.named_scope("phase_AKV"),
            tc.tile_pool(name="wkp", bufs=1) as wkp,
            tc.tile_pool(name="wvp", bufs=1) as wvp,
            tc.tile_pool(name="xtp", bufs=2) as xtp,
            tc.tile_pool(name="kst", bufs=3) as kstp,
            tc.tile_pool(name="pk", bufs=3, space="PSUM") as pkp,
            tc.tile_pool(name="pv", bufs=3, space="PSUM") as pvp,
        ):
            wk_t = [wkp.tile([128, 4, DG], BF16, name=f"wk{d}", tag=f"wk{d}") for d in range(4)]
            wv_t = [wvp.tile([128, 4, DG], BF16, name=f"wv{d}", tag=f"wv{d}") for d in range(4)]
            for d in range(4):
                nc.sync.dma_start(wk_t[d][:], wkr[:, 4 * d : 4 * d + 4, :])
            for d in range(4):
                nc.sync.dma_start(wv_t[d][:], wvr[:, 4 * d : 4 * d + 4, :])
            for ts in range(T // KTS):
                xt_t = [xtp.tile([128, 4, KTS], BF16, name=f"xt{d}", tag=f"xt{d}") for d in range(4)]
                for d in range(4):
                    nc.sync.dma_start(
                        xt_t[d][:], xTr[:, 4 * d : 4 * d + 4, ts * KTS : (ts + 1) * KTS]
                    )
                # K^T: per head j, (hd, KTS) tile; staged 4 heads per DMA
                for jg in range(2):
                    ks = kstp.tile([128, 4, KTS], BF16, name="ks", tag="ks")
                    for jj in range(4):
                        j = 4 * jg + jj
                        pk = pkp.tile([128, KTS], F32, name="pk", tag="pk")
                        for d in range(ND):
                            nc.tensor.matmul(
                                pk[:],
                                wk_t[d // 4][:, d % 4, j * 128 : (j + 1) * 128],
                                xt_t[d // 4][:, d % 4, :],
                                start=(d == 0),
                                stop=(d == ND - 1),
                            )
                        nc.scalar.activation(
                            kt0_sb[:, ts * KTS : (ts + 1) * KTS] if j == 0
                            else ks[:, jj, :],
                            pk[:],
                            mybir.ActivationFunctionType.Identity,
                            bias=bias_k[:, j : j + 1],
                        )
                    nc.sync.dma_start(
                        ktd_r[:, 4 * jg : 4 * jg + 4, ts * KTS : (ts + 1) * KTS],
                        ks[:],
                    )
                # V: (t, DG) tiles
                for u in range(KTS // 128):
                    i = ts * (KTS // 128) + u
                    for f in range(2):
                        pv = pvp.tile([128, 512], F32, name="pv", tag="pv")
                        for d in range(ND):
                            nc.tensor.matmul(
                                pv[:],
                                xt_t[d // 4][:, d % 4, u * 128 : (u + 1) * 128],
                                wv_t[d // 4][:, d % 4, f * 512 : (f + 1) * 512],
                                start=(d == 0),
                                stop=(d == ND - 1),
                            )
                        nc.vector.tensor_copy(
                            v_t[i][:, f * 512 : (f + 1) * 512], pv[:]
                        )

        # ---- phase B prefetch: out-proj weights + masks ------------
        wop = ctx.enter_context(tc.tile_pool(name="wop", bufs=1))
        wo_t = [wop.tile([128, 4, D], BF16, name=f"wo{d}", tag=f"wo{d}") for d in range(2)]
        for d in range(2):
            nc.sync.dma_start(wo_t[d][:], wor[:, 4 * d : 4 * d + 4, :])
        for g in range(NT // 4):
            if mlo[g] < M and mhi[g] > mlo[g]:
                nc.sync.dma_start(
                    mask_t[g][:, :, : mhi[g] - mlo[g]],
                    maskr[:, 4 * g : 4 * g + 4, mlo[g] : mhi[g]],
                )

        # ---- phase B: attention per head, 4-chunk batched ----------
        chunks = [i for i in range(NT) if flo[i] < M]
        pairs = [chunks[k : k + 2] for k in range(0, len(chunks), 2)]
        batches = [pairs[k : k + 2] for k in range(0, len(pairs), 2)]
        with (
            nc.named_scope("phase_B"),
            tc.tile_pool(name="kth", bufs=2) as kthp,
            tc.tile_pool(name="ps", bufs=2, space="PSUM") as psp,
            tc.tile_pool(name="po", bufs=2, space="PSUM") as pop,
            tc.tile_pool(name="pl", bufs=2, space="PSUM") as plp,
            tc.tile_pool(name="esb", bufs=5) as esb,
            tc.tile_pool(name="lsb", bufs=2) as lsb,
        ):
            po_q, pl_q = {}, {}

            def emit_norm(j):
                """Normalize head j: ot[j] = po[j] / l[j] (off PE critical path).

                l is broadcast across partitions on the PE first, so the
                reciprocal runs 128 lanes wide instead of on 1 partition.
                """
                po, pl = po_q.pop(j), pl_q.pop(j)
                l_sb = lsb.tile([1, M], F32, name="l", tag="l")
                linv = lsb.tile([1, M], F32, name="linv", tag="linv")
                nc.vector.tensor_copy(l_sb[:], pl[:])
                nc.vector.reciprocal(linv[:], l_sb[:])
                pb = psp.tile([128, M], F32, name="pb", tag="ps")
                nc.tensor.matmul(pb[:], ones_r[:], linv[:], start=True, stop=True)
                lb_sb = lsb.tile([128, M], F32, name="lb", tag="lb")
                nc.scalar.copy(lb_sb[:], pb[:])
                nc.vector.tensor_mul(ot_t[j][:], po[:], lb_sb[:])

            kth = {}
            for j in range(NHG):
                if j == 0:
                    kth[0] = kt0_sb
                po = pop.tile([128, M], F32, name="po", tag="po")
                pl = plp.tile([1, M], F32, name="pl", tag="pl")
                po_q[j], pl_q[j] = po, pl
                pend = None  # previous batch: list of (i, e, u, lo)
                first = True
                lfirst = True
                for bi, batch in enumerate(batches):
                    cur = []
                    for pair in batch:
                        lo_min = min(flo[i] for i in pair)
                        pst = psp.tile([128, 2, M], F32, name="pst", tag="ps")
                        for u, i in enumerate(pair):
                            nc.tensor.matmul(
                                pst[:, u, flo[i] : M],
                                kth[j][:, i * 128 : (i + 1) * 128],
                                qt_t[j][:, flo[i] : M],
                                start=True,
                                stop=True,
                                skip_group_check=True,
                            )
                        # one mask add + one exp per pair; mask cols beyond a
                        # chunk's own [lo, fhi) window add 0 or touch lanes
                        # the narrower chunk never reads
                        fhi_max = max(fhi[i] for i in pair)
                        g = pair[0] // 4
                        um = pair[0] % 4
                        if lo_min < fhi_max:
                            nc.vector.tensor_add(
                                pst[:, : len(pair), lo_min:fhi_max],
                                pst[:, : len(pair), lo_min:fhi_max],
                                mask_t[g][:, um : um + len(pair), lo_min - mlo[g] : fhi_max - mlo[g]],
                            )
                        e = esb.tile([128, 2, M], BF16, name="e", tag="e")
                        nc.scalar.activation(
                            e[:, : len(pair), lo_min:M],
                            pst[:, : len(pair), lo_min:M],
                            mybir.ActivationFunctionType.Exp,
                            bias=zbias[:],
                        )
                        for u, i in enumerate(pair):
                            cur.append((i, e, u, flo[i]))
                    if pend:
                        for i, e, u, lo in pend:
                            nc.tensor.matmul(
                                po[:, lo:M],
                                v_t[i][:, j * 128 : (j + 1) * 128],
                                e[:, u, lo:M],
                                start=first, stop=False, skip_group_check=True,
                            )
                            first = False
                        for i, e, u, lo in pend:
                            nc.tensor.matmul(
                                pl[:, lo:M], ones_c[:], e[:, u, lo:M],
                                start=lfirst, stop=False, skip_group_check=True,
                            )
                            lfirst = False
                    pend = cur
                    if bi == 2:
                        # prefetch next head's K^T; emit previous head's norm
                        if j + 1 < NHG:
                            kth[j + 1] = kthp.tile([128, T], BF16, name="kth", tag="kth")
                            nc.sync.dma_start(kth[j + 1][:], ktd[j + 1])
                        if j > 0 and (j - 1) in po_q:
                            emit_norm(j - 1)
                for k, (i, e, u, lo) in enumerate(pend):
                    last = k == len(pend) - 1
                    nc.tensor.matmul(
                        po[:, lo:M],
                        v_t[i][:, j * 128 : (j + 1) * 128],
                        e[:, u, lo:M],
                        start=first, stop=last, skip_group_check=True,
                    )
                    first = False
                for k, (i, e, u, lo) in enumerate(pend):
                    nc.tensor.matmul(
                        pl[:, lo:M], ones_c[:], e[:, u, lo:M],
                        start=lfirst, stop=(k == len(pend) - 1),
                        skip_group_check=True,
                    )
                    lfirst = False
                kth.pop(j, None)
                if j > 0 and (j - 1) in po_q:
                    emit_norm(j - 1)
            emit_norm(NHG - 1)

        # ---- phase C: y = O @ wo  (row-parallel partial) -----------
        with (
            nc.named_scope("phase_C"),
            tc.tile_pool(name="py", bufs=2, space="PSUM") as pyp,
            tc.tile_pool(name="ysb", bufs=3) as ysb,
        ):
            # fo pairs share the stationary ot slice -> one weight load
            # feeds two 512-wide matmuls
            for mb in range(M // 128):
                for fp in range(D // 1024):
                    py = [
                        pyp.tile([128, 512], F32, name="py", tag=f"py{h}")
                        for h in range(2)
                    ]
                    for j in range(NHG):
                        for h in range(2):
                            fo = 2 * fp + h
                            nc.tensor.matmul(
                                py[h][:],
                                ot_t[j][:, mb * 128 : (mb + 1) * 128],
                                wo_t[j // 4][:, j % 4, fo * 512 : (fo + 1) * 512],
                                start=(j == 0),
                                stop=(j == NHG - 1),
                                skip_group_check=True,
                            )
                    for h in range(2):
                        ys = ysb.tile([128, 512], F32, name="ys", tag="ys")
                        nc.scalar.copy(ys[:], py[h][:])
                        nc.sync.dma_start(
                            y[
                                mb * 128 : (mb + 1) * 128,
                                (2 * fp + h) * 512 : (2 * fp + h + 1) * 512,
                            ],
                            ys[:],
                        )

    nc.compile()
    return nc


_cache = {}


def _get_program(flo, fhi):
    key = (tuple(flo), tuple(fhi))
    if key not in _cache:
        _cache[key] = build_program(list(flo), list(fhi))
    return _cache[key]


def _prep(inputs):
    x = np.asarray(inputs["x"], dtype=np.float32)
    qidx = np.asarray(inputs["query_idx"]).astype(np.int64)
    Wq = np.asarray(inputs["Wq"], dtype=np.float32)
    Wk = np.asarray(inputs["Wk"], dtype=np.float32)
    Wv = np.asarray(inputs["Wv"], dtype=np.float32)
    Wo = np.asarray(inputs["Wo"], dtype=np.float32)
    bq = np.asarray(inputs["bq"], dtype=np.float32)
    bk = np.asarray(inputs["bk"], dtype=np.float32)
    bv = np.asarray(inputs["bv"], dtype=np.float32)
    bo = np.asarray(inputs["bo"], dtype=np.float32)

    # Per-t-chunk skip bounds, union over batches.  flo[i] = first m that
    # attends into chunk i (everything below is fully masked there);
    # fhi[i] = one past the last m only partially covered by chunk i.
    # Computed positionally so they are correct even for unsorted
    # query_idx (just less effective at skipping).
    flo = [M] * NT
    fhi = [0] * NT
    for b in range(B):
        for i in range(NT):
            allowed = qidx[b] >= 128 * i          # chunk i not fully masked
            partial = qidx[b] < 128 * (i + 1)     # chunk i not fully allowed
            lo_b = int(np.argmax(allowed)) if allowed.any() else M
            hi_b = M - int(np.argmax(partial[::-1])) if partial.any() else 0
            flo[i] = min(flo[i], lo_b)
            fhi[i] = max(fhi[i], hi_b)

    in_maps = []
    tgrid = np.arange(T)[:, None]
    for core in range(8):
        b, g = divmod(core, 2)
        sl = slice(g * DG, (g + 1) * DG)
        xb = x[b]
        mask = np.where(tgrid <= qidx[b][None, :], np.float32(0), MASK_VAL)
        in_maps.append(
            {
                "xT": np.ascontiguousarray(xb.T.astype(NPBF)),
                "xqT": np.ascontiguousarray(xb[qidx[b]].T.astype(NPBF)),
                "wk": np.ascontiguousarray(Wk[:, sl].astype(NPBF)),
                "wv": np.ascontiguousarray(Wv[:, sl].astype(NPBF)),
                "wq": np.ascontiguousarray(Wq[:, sl].astype(NPBF)),
                "wo": np.ascontiguousarray(Wo[sl, :].astype(NPBF)),
                "mask": np.ascontiguousarray(mask.astype(NPBF)),
                "bks": np.ascontiguousarray(bk[sl].reshape(NHG, 128).T),
                "bqs": np.ascontiguousarray(
                    (bq[sl] / np.sqrt(HD)).reshape(NHG, 128).T.astype(np.float32)
                ),
            }
        )

    const = (bv.astype(np.float64) @ Wo.astype(np.float64) + bo).astype(np.float32)
    return flo, fhi, in_maps, const


def run(inputs, trace=False, trace_kwargs=None):
    _install_ntff_hook()
    flo, fhi, in_maps, const = _prep(inputs)
    nc = _get_program(flo, fhi)
    res = run_bass_kernel_spmd(
        nc, in_maps, list(range(8)), trace=trace, **(trace_kwargs or {})
    )
    out = np.zeros((B, M, D), dtype=np.float32)
    for b in range(B):
        out[b] = res.results[2 * b]["y"] + res.results[2 * b + 1]["y"] + const
    return out, res


def kernel(**inputs) -> np.ndarray:
    out, _ = run(inputs, trace=False)
    return out
